# revision 1
# baseline (speedup 1.0000x reference)
"""Trainium2 Bass kernel for nn_AttentionLayer (conv1d -> linear attention -> gelu + residual).

Full inputs:  x [8, 256, 4096] f32, conv_w [512, 256, 3] f32, conv_b [512] f32
Full output:  [8, 256, 4096] f32

Sharding: pure data-parallel over batch B=8 -> 8 NeuronCores, one batch each.
No collectives needed.

Per-core math (C=256, N=4096, one batch):
  y    = conv1d(x, w, pad=1) + b          # [2C, N]
  q    = phi(y[:C]),  k = phi(y[C:])      # phi = elu+1 = max(y+1, exp(min(y,0)))
  v    = x^T                              # [N, C]
  kv   = sum_n phi(k)[n,:] (x) v[n,:]     # [C, C]
  out  = gelu(q @ kv) + x                 # [C, N]

Layout trick: the conv contraction (over input channels ci) lets us produce
q in [c, n] layout (w^T as stationary operand) AND k in [n, c] layout
(x as stationary operand) with zero transposes. v^T (= x^T) is shipped
pre-transposed/pre-cast from the host, as are bf16 copies of x and w.

Matmuls run in bf16 (f32 PSUM accumulate): bf16 gets pipelined LDWEIGHTS
(f32/f32r matmuls serialize a ~107ns self-weight-load per matmul).
phi is 3 ops via the fused scalar_tensor_tensor: min (DVE) ->
exp (ACT, one table per phase) -> (y+1) max e (DVE). The conv bias for
the k half enters as a rank-1 start matmul (ones^T @ b_k); for the q half
it rides the DVE ops' per-partition scalar operand. Residual add uses an
f32 copy of x on GpSimd (the only engine with slack).
"""

import ml_dtypes
import numpy as np

import concourse.bass as bass
import concourse.mybir as mybir
import concourse.tile as tile
from concourse import bacc
from concourse.bass_utils import run_bass_kernel_spmd

F32 = mybir.dt.float32
BF16 = mybir.dt.bfloat16
AF = mybir.ActivationFunctionType
ALU = mybir.AluOpType

B, C, N = 8, 256, 4096
NCORES = 8
CT = C // 128        # 2 c-tiles (partition groups) per 256-channel dim
NJ = N // 512        # 8 column chunks of 512
NT = N // 128        # 32 n-tiles of 128
NP = N + 2           # x padded with one zero column on each side

BF = ml_dtypes.bfloat16


def _build_nc():
    nc = bacc.Bacc("TRN2", target_bir_lowering=False, debug=False, num_devices=NCORES)

    xb_d = nc.declare_dram_parameter("xb", [C, NP], BF16, isOutput=False)
    xt_d = nc.declare_dram_parameter("xt", [N, C], BF16, isOutput=False)
    wt_d = nc.declare_dram_parameter("wt", [3, CT, 128, 512], BF16, isOutput=False)
    bq_d = nc.declare_dram_parameter("bq", [CT, 128, 1], F32, isOutput=False)
    bq1_d = nc.declare_dram_parameter("bq1", [CT, 128, 1], F32, isOutput=False)
    bk_d = nc.declare_dram_parameter("bk", [2, 256], BF16, isOutput=False)
    out_d = nc.declare_dram_parameter("out", [C, N], F32, isOutput=True)

    with tile.TileContext(nc) as tc:
        with (
            tc.tile_pool(name="persist", bufs=1) as per,
            tc.tile_pool(name="tmp", bufs=6) as tmp,
            tc.tile_pool(name="psum", bufs=6, space="PSUM") as ps,
            tc.tile_pool(name="psum2", bufs=2, space="PSUM") as ps2,
        ):
            # ---- constants / weights -------------------------------------
            ones = per.tile([1, 128], BF16, tag="ones")
            nc.sync.dma_start(out=ones, in_=bk_d[0:1, 0:128])
            bk_sb = per.tile([1, 256], BF16, tag="bk")
            nc.sync.dma_start(out=bk_sb, in_=bk_d[1:2, :])
            bq_sb = per.tile([128, CT, 1], F32, tag="bq")
            bq1_sb = per.tile([128, CT, 1], F32, tag="bq1")
            for ct in range(CT):
                nc.sync.dma_start(out=bq_sb[:, ct, :], in_=bq_d[ct, :, :])
                nc.sync.dma_start(out=bq1_sb[:, ct, :], in_=bq1_d[ct, :, :])

            wt_sb = [[per.tile([128, 512], BF16, tag=f"wt{t}{ci}", name=f"wt{t}{ci}")
                      for ci in range(CT)] for t in range(3)]
            for t in range(3):
                for ci in range(CT):
                    nc.sync.dma_start(out=wt_sb[t][ci][:, 256:512],
                                      in_=wt_d[t, ci, :, 256:512])

            # bf16 x chunks for the conv matmuls; chunk 0 first (head)
            xs = [[per.tile([128, 514], BF16, tag=f"x{ci}{j}", name=f"x{ci}{j}")
                   for j in range(NJ)] for ci in range(CT)]
            for j in range(NJ):
                for ci in range(CT):
                    nc.sync.dma_start(
                        out=xs[ci][j],
                        in_=xb_d[ci * 128:(ci + 1) * 128, j * 512:j * 512 + 514],
                    )
            for t in range(3):
                for ci in range(CT):
                    nc.sync.dma_start(out=wt_sb[t][ci][:, 0:256],
                                      in_=wt_d[t, ci, :, 0:256])

            # v^T straight from DRAM (host pre-transposed bf16)
            vT = per.tile([128, NT, 256], BF16, tag="vT")
            nc.sync.dma_start(
                out=vT, in_=xt_d.rearrange("(i p) d -> p i d", p=128))

            # ---- persistent intermediates --------------------------------
            kT = per.tile([128, NT, 256], BF16, tag="kT")    # phi(k) in [n, c]
            qphi = [per.tile([128, N], BF16, tag=f"qphi{ct}", name=f"qphi{ct}")
                    for ct in range(CT)]
            kv_sb = per.tile([128, CT, 256], BF16, tag="kv")  # kv in [c, d]

            # ---- phase NT: k^T (conv in transposed layout) ---------------
            for i in range(NT):
                j, off = i // 4, (i % 4) * 128
                kt_ps = ps.tile([128, 512], F32, tag="bank", name="kt_ps")
                # bias row: ones^T @ bk broadcasts conv_b[k-half] over rows
                kt_ps = kt_ps[:, 0:256]
                nc.tensor.matmul(kt_ps, ones, bk_sb, start=True, stop=False)
                for ci in range(CT):
                    for t in range(3):
                        nc.tensor.matmul(
                            kt_ps,
                            xs[ci][j][:, off + t:off + t + 128],
                            wt_sb[t][ci][:, 256:512],
                            start=False,
                            stop=(ci == CT - 1 and t == 2),
                        )
                # phi: kT = max(y+1, exp(min(y, 0)))
                tmin = tmp.tile([128, 256], F32, tag="ntmin")
                nc.vector.tensor_scalar(tmin, kt_ps, 0.0, None, ALU.min)
                e = tmp.tile([128, 256], F32, tag="nte")
                nc.scalar.activation(e, tmin, AF.Exp)
                nc.vector.scalar_tensor_tensor(
                    kT[:, i, :], kt_ps, 1.0, e, ALU.add, ALU.max)

            # ---- phase Q: conv q in [c, n] layout ------------------------
            for ct in range(CT):
                for j in range(NJ):
                    q_ps = ps.tile([128, 512], F32, tag="bank", name="q_ps")
                    first = True
                    for ci in range(CT):
                        for t in range(3):
                            nc.tensor.matmul(
                                q_ps,
                                wt_sb[t][ci][:, ct * 128:(ct + 1) * 128],
                                xs[ci][j][:, t:t + 512],
                                start=first,
                                stop=(ci == CT - 1 and t == 2),
                            )
                            first = False
                    # phi with per-partition conv bias folded in:
                    #   min(y+b, 0) then (y + (b+1)) max exp(...)
                    tmin = tmp.tile([128, 512], F32, tag="qtmin")
                    nc.vector.tensor_scalar(
                        tmin, q_ps, bq_sb[:, ct, :], 0.0, ALU.add, ALU.min)
                    e = tmp.tile([128, 512], F32, tag="qte")
                    nc.scalar.activation(e, tmin, AF.Exp)
                    nc.vector.scalar_tensor_tensor(
                        qphi[ct][:, j * 512:(j + 1) * 512],
                        q_ps, bq1_sb[:, ct, :], e, ALU.add, ALU.max)

            # ---- phase KV: kv[c, d] = sum_n k^T[n, c] v^T[n, d] ----------
            for ch in range(CT):
                kv_ps = ps2.tile([128, 256], F32, tag="kvp", name="kv_ps")
                for i in range(NT):
                    nc.tensor.matmul(
                        kv_ps,
                        kT[:, i, ch * 128:(ch + 1) * 128],
                        vT[:, i, :],
                        start=(i == 0),
                        stop=(i == NT - 1),
                    )
                nc.scalar.copy(kv_sb[:, ch, :], kv_ps)

            # ---- phase OUT: out[d, n] = gelu(sum_c kv[c, d] q[c, n]) + x -
            for dt in range(CT):
                for j in range(NJ):
                    o_ps = ps.tile([128, 512], F32, tag="bank", name="o_ps")
                    for ch in range(CT):
                        nc.tensor.matmul(
                            o_ps,
                            kv_sb[:, ch, dt * 128:(dt + 1) * 128],
                            qphi[ch][:, j * 512:(j + 1) * 512],
                            start=(ch == 0),
                            stop=(ch == CT - 1),
                        )
                    g = tmp.tile([128, 512], F32, tag="og")
                    nc.scalar.activation(g, o_ps, AF.Gelu)
                    o = tmp.tile([128, 512], F32, tag="oo")
                    nc.gpsimd.tensor_add(o, g, xs[dt][j][:, 1:513])
                    nc.sync.dma_start(
                        out=out_d[dt * 128:(dt + 1) * 128, j * 512:(j + 1) * 512],
                        in_=o,
                    )

    nc.compile()
    return nc


_NC_CACHE = None


def _get_nc():
    global _NC_CACHE
    if _NC_CACHE is None:
        _NC_CACHE = _build_nc()
    return _NC_CACHE


def _prep(x, conv_w, conv_b):
    x = np.asarray(x, dtype=np.float32)
    conv_w = np.asarray(conv_w, dtype=np.float32)
    conv_b = np.asarray(conv_b, dtype=np.float32)
    xp = np.zeros((B, C, NP), dtype=BF)
    xp[:, :, 1:N + 1] = x.astype(BF)
    xt = np.ascontiguousarray(x.transpose(0, 2, 1)).astype(BF)   # [B, N, C]
    # wt[t, ci_tile, ci, co] = conv_w[co, ci_tile*128 + ci, t]
    wt = np.ascontiguousarray(
        conv_w.transpose(2, 1, 0).reshape(3, CT, 128, 2 * C)).astype(BF)
    bq = np.ascontiguousarray(conv_b[:C].reshape(CT, 128, 1))
    bq1 = np.ascontiguousarray(bq + 1.0)
    bk = np.ones((2, C), dtype=np.float32)
    bk[1, :] = conv_b[C:]
    bk = np.ascontiguousarray(bk).astype(BF)
    return xp, xt, wt, bq, bq1, bk


def make_in_maps(x, conv_w, conv_b):
    xp, xt, wt, bq, bq1, bk = _prep(x, conv_w, conv_b)
    return [
        {"xb": xp[b], "xt": xt[b], "wt": wt,
         "bq": bq, "bq1": bq1, "bk": bk}
        for b in range(B)
    ]


def kernel(x: np.ndarray, conv_w: np.ndarray, conv_b: np.ndarray) -> np.ndarray:
    nc = _get_nc()
    in_maps = make_in_maps(x, conv_w, conv_b)
    res = run_bass_kernel_spmd(nc, in_maps, core_ids=list(range(NCORES)))
    return np.stack([res.results[b]["out"] for b in range(B)], axis=0)



# revision 23
# speedup vs baseline: 2.1731x; 2.1731x over previous
"""Trainium2 Bass kernel for nn_AttentionLayer (conv1d -> linear attention -> gelu + residual).

Full inputs:  x [8, 256, 4096] f32, conv_w [512, 256, 3] f32, conv_b [512] f32
Full output:  [8, 256, 4096] f32

Sharding: pure data-parallel over batch B=8 -> 8 NeuronCores, one batch each.
No collectives needed.

Per-core math (C=256, N=4096, one batch):
  y    = conv1d(x, w, pad=1) + b          # [2C, N]
  q    = phi(y[:C]),  k = phi(y[C:])      # phi = elu+1 = max(y+1, exp(min(y,0)))
  v    = x^T                              # [N, C]
  kv   = sum_n phi(k)[n,:] (x) v[n,:]     # [C, C]
  out  = gelu(q @ kv) + x                 # [C, N]

Layout trick: the conv contraction (over input channels ci) lets us produce
q in [c, n] layout (w^T as stationary operand) AND k in [n, c] layout
(x as stationary operand) with zero transposes. v^T (= x^T) is shipped
pre-transposed/pre-tiled/pre-cast from the host, as are bf16 copies of x
and w.

DMA hygiene (the real-HW bottleneck): every HWDGE dma_start costs ~650ns of
serialized ring dispatch, and scattered patterns fragment into 512B
descriptors (the naive v^T load alone was ~0.5ms on HW). So all inputs are
host-packed into layouts that give >=1KB-per-partition contiguous rows and
are loaded with a handful of large DMAs on the SP ring, while v^T and the
output stores ride the ACT ring.

Matmuls run in bf16 (f32 PSUM accumulate): bf16 gets pipelined LDWEIGHTS
(f32/f32r matmuls serialize a ~107ns self-weight-load per matmul).
phi is 3 ops via the fused scalar_tensor_tensor: min (DVE) ->
exp (ACT, one table per phase) -> (y+1) max e (DVE). The conv bias for
the k half enters as a rank-1 start matmul (ones^T @ b_k); for the q half
it rides the DVE ops' per-partition scalar operand. Residual add runs on
DVE (Pool/gpsimd serialized the OUT-phase tail at ~1.1us per add).
"""

import ml_dtypes
import numpy as np

import concourse.bass as bass
import concourse.mybir as mybir
import concourse.tile as tile
from concourse import bacc
from concourse.bass_utils import run_bass_kernel_spmd

F32 = mybir.dt.float32
BF16 = mybir.dt.bfloat16
AF = mybir.ActivationFunctionType
ALU = mybir.AluOpType

B, C, N = 8, 256, 4096
NCORES = 8
CT = C // 128        # 2 c-tiles (partition groups) per 256-channel dim
NJ = N // 512        # 8 column chunks of 512
NT = N // 128        # 32 n-tiles of 128
NP = N + 2           # x padded with one zero column on each side
NH = NP // 2 + 1     # xs half-load split point (2050)

BF = ml_dtypes.bfloat16


def _build_nc(reps=1, hw_loop=False):
    nc = bacc.Bacc("TRN2", target_bir_lowering=False, debug=False, num_devices=NCORES)

    # Host-packed parameter layouts (see _prep):
    #  xb   [CT, 128, NP]    bf16  x padded, partition-tiled over channels
    #  xt   [128, NT*256]    bf16  v^T tiled: [p, i*256+d] = x[d, i*128+p]
    #  wtk  [128, 6, 256]    bf16  conv w, k-half; slot ci*3+t, partition=cin
    #  wtq  [128, 6, 256]    bf16  conv w, q-half
    #  bqa  [128, 2*CT]      f32   [conv_b_q | conv_b_q + 1] partition-tiled
    #  obk  [1, 384]         bf16  [ones(128) | conv_b_k(256)]
    xb_d = nc.declare_dram_parameter("xb", [CT, 128, NP], BF16, isOutput=False)
    xt_d = nc.declare_dram_parameter("xt", [128, NT * 256], BF16, isOutput=False)
    wtk_d = nc.declare_dram_parameter("wtk", [128, 6 * 256], BF16, isOutput=False)
    wtq_d = nc.declare_dram_parameter("wtq", [128, 6 * 256], BF16, isOutput=False)
    bqa_d = nc.declare_dram_parameter("bqa", [128, 2 * CT], F32, isOutput=False)
    obk_d = nc.declare_dram_parameter("obk", [1, 384], BF16, isOutput=False)
    out_d = nc.declare_dram_parameter("out", [C, N], BF16, isOutput=True)

    with tile.TileContext(nc) as tc:
        with (
            tc.tile_pool(name="persist", bufs=1) as per,
            tc.tile_pool(name="tmp", bufs=6) as tmp,
            tc.tile_pool(name="obuf", bufs=3) as obuf,
            tc.tile_pool(name="psum", bufs=6, space="PSUM") as ps,
            tc.tile_pool(name="psum2", bufs=2, space="PSUM") as ps2,
        ):
          import contextlib
          loop_ctx = tc.For_i(0, reps, 1) if hw_loop else contextlib.nullcontext()
          with loop_ctx:
           for _rep in range(1 if hw_loop else reps):
            # ---- inputs: few large DMAs; SP ring for conv-critical ones ---
            obk = per.tile([1, 384], BF16, tag="obk", name="obk")
            nc.sync.dma_start(out=obk, in_=obk_d[:, :])
            ones = obk[0:1, 0:128]
            bk_sb = obk[0:1, 128:384]
            # Warm the ACT Exp table while the bulk DMAs land: the first
            # phi chain otherwise eats the ~1.3us LoadActFuncSet. (Must
            # read initialized SBUF - a memzero'd scratch tile faults the
            # exec unit.)
            warm = tmp.tile([1, 1], F32, tag="warm", name="warm")
            nc.scalar.activation(warm, obk[0:1, 0:1], AF.Exp)

            wtk = per.tile([128, 6, 256], BF16, tag="wtk", name="wtk")
            wtk_r = wtk_d.rearrange("p (s c) -> p s c", c=256)
            nc.sync.dma_start(out=wtk[:, 0:3, :], in_=wtk_r[:, 0:3, :])

            # x in [c, n] layout: one tile per ci, staged loads (small
            # leading chunk per ci so the conv starts ASAP, then the rest).
            NQ = 514
            xs = [per.tile([128, NP], BF16, tag=f"x{ci}", name=f"x{ci}")
                  for ci in range(CT)]
            nc.sync.dma_start(out=xs[0][:, 0:NQ], in_=xb_d[0, :, 0:NQ])
            nc.sync.dma_start(out=wtk[:, 3:6, :], in_=wtk_r[:, 3:6, :])
            nc.sync.dma_start(out=xs[1][:, 0:NQ], in_=xb_d[1, :, 0:NQ])
            for ci in range(CT):
                nc.sync.dma_start(out=xs[ci][:, NQ:NH], in_=xb_d[ci, :, NQ:NH])
            bqa = per.tile([128, 2 * CT], F32, tag="bqa", name="bqa")
            nc.sync.dma_start(out=bqa, in_=bqa_d[:, :])
            for ci in range(CT):
                nc.sync.dma_start(out=xs[ci][:, NH:NP], in_=xb_d[ci, :, NH:NP])

            wtq = per.tile([128, 6, 256], BF16, tag="wtq", name="wtq")
            nc.sync.dma_start(out=wtq, in_=wtq_d.rearrange("p (s c) -> p s c", c=256))

            # v^T last: not needed until phase KV, and its 2MB transfer
            # must not delay the conv-critical loads above.
            vT = per.tile([128, NT, 256], BF16, tag="vT", name="vT")
            nc.sync.dma_start(
                out=vT, in_=xt_d.rearrange("p (i d) -> p i d", d=256))

            # ---- persistent intermediates --------------------------------
            kT = per.tile([128, NT, 256], BF16, tag="kT", name="kT")
            qphi = [per.tile([128, N], BF16, tag=f"qphi{ct}", name=f"qphi{ct}")
                    for ct in range(CT)]
            kv_sb = per.tile([128, CT, 256], BF16, tag="kv", name="kv_sb")

            # ---- phase NT: k^T (conv in transposed layout) ---------------
            for i in range(NT):
                off = i * 128
                kt_ps = ps.tile([128, 512], F32, tag="bank", name="kt_ps")
                # bias row: ones^T @ bk broadcasts conv_b[k-half] over rows
                # (walrus rejects non-PE engines preloading PSUM, so this
                # stays a rank-1 start matmul)
                kt_ps = kt_ps[:, 0:256]
                nc.tensor.matmul(kt_ps, ones, bk_sb, start=True, stop=False)
                for ci in range(CT):
                    for t in range(3):
                        nc.tensor.matmul(
                            kt_ps,
                            xs[ci][:, off + t:off + t + 128],
                            wtk[:, ci * 3 + t, :],
                            start=False,
                            stop=(ci == CT - 1 and t == 2),
                        )
                # phi: kT = max(y+1, exp(min(y, 0)))
                tmin = tmp.tile([128, 256], F32, tag="ntmin", name="tmin_nt")
                nc.vector.tensor_scalar(tmin, kt_ps, 0.0, None, ALU.min)
                e = tmp.tile([128, 256], F32, tag="nte", name="e_nt")
                nc.scalar.activation(e, tmin, AF.Exp)
                nc.vector.scalar_tensor_tensor(
                    kT[:, i, :], kt_ps, 1.0, e, ALU.add, ALU.max)

            # ---- phase Q: conv q in [c, n] layout ------------------------
            for ct in range(CT):
                for j in range(NJ):
                    q_ps = ps.tile([128, 512], F32, tag="bank", name="q_ps")
                    first = True
                    for ci in range(CT):
                        for t in range(3):
                            nc.tensor.matmul(
                                q_ps,
                                wtq[:, ci * 3 + t, ct * 128:(ct + 1) * 128],
                                xs[ci][:, j * 512 + t:j * 512 + t + 512],
                                start=first,
                                stop=(ci == CT - 1 and t == 2),
                            )
                            first = False
                    # phi with per-partition conv bias folded in:
                    #   min(y+b, 0) then (y + (b+1)) max exp(...)
                    tmin = tmp.tile([128, 512], F32, tag="qtmin", name="tmin_q")
                    nc.vector.tensor_scalar(
                        tmin, q_ps, bqa[:, ct:ct + 1], 0.0, ALU.add, ALU.min)
                    e = tmp.tile([128, 512], F32, tag="qte", name="e_q")
                    nc.scalar.activation(e, tmin, AF.Exp)
                    nc.vector.scalar_tensor_tensor(
                        qphi[ct][:, j * 512:(j + 1) * 512],
                        q_ps, bqa[:, CT + ct:CT + ct + 1], e, ALU.add, ALU.max)

            # ---- phase KV: kv[c, d] = sum_n k^T[n, c] v^T[n, d] ----------
            for ch in range(CT):
                kv_ps = ps2.tile([128, 256], F32, tag="kvp", name="kv_ps")
                for i in range(NT):
                    nc.tensor.matmul(
                        kv_ps,
                        kT[:, i, ch * 128:(ch + 1) * 128],
                        vT[:, i, :],
                        start=(i == 0),
                        stop=(i == NT - 1),
                    )
                nc.vector.tensor_copy(kv_sb[:, ch, :], kv_ps)

            # ---- phase OUT: out[d, n] = gelu(sum_c kv[c, d] q[c, n]) + x -
            # Stores batched in 1024-col pairs: halves the HWDGE dispatches
            # while keeping the tail transfer small.
            for dt in range(CT):
                ob = None
                for j in range(NJ):
                    o_ps = ps.tile([128, 512], F32, tag="bank", name="o_ps")
                    for ch in range(CT):
                        nc.tensor.matmul(
                            o_ps,
                            kv_sb[:, ch, dt * 128:(dt + 1) * 128],
                            qphi[ch][:, j * 512:(j + 1) * 512],
                            start=(ch == 0),
                            stop=(ch == CT - 1),
                        )
                    g = tmp.tile([128, 512], BF16, tag="og", name="g_out")
                    nc.scalar.activation(g, o_ps, AF.Gelu)
                    if j % 2 == 0:
                        ob = obuf.tile([128, 1024], BF16, tag="ob", name="ob")
                    nc.vector.tensor_tensor(
                        ob[:, (j % 2) * 512:(j % 2) * 512 + 512],
                        g, xs[dt][:, j * 512 + 1:j * 512 + 513], ALU.add)
                    if j % 2 == 1:
                        nc.sync.dma_start(
                            out=out_d[dt * 128:(dt + 1) * 128,
                                      (j - 1) * 512:(j + 1) * 512],
                            in_=ob,
                        )

    nc.compile()
    return nc


_NC_CACHE = None


def _get_nc():
    global _NC_CACHE
    if _NC_CACHE is None:
        _NC_CACHE = _build_nc()
    return _NC_CACHE


def _prep(x, conv_w, conv_b):
    x = np.asarray(x, dtype=np.float32)
    conv_w = np.asarray(conv_w, dtype=np.float32)
    conv_b = np.asarray(conv_b, dtype=np.float32)
    xb = np.zeros((B, CT, 128, NP), dtype=BF)
    xb[:, :, :, 1:N + 1] = x.reshape(B, CT, 128, N).astype(BF)
    # xt[b, p, i*256 + d] = x[b, d, i*128 + p]  (v^T tiled for contiguous DMA)
    xt = np.ascontiguousarray(
        x.transpose(0, 2, 1).reshape(B, NT, 128, C).transpose(0, 2, 1, 3)
    ).reshape(B, 128, NT * C).astype(BF)
    # w[t, ci_t, p, co] = conv_w[co, ci_t*128 + p, t]; slot = ci*3 + t
    w = conv_w.transpose(2, 1, 0).reshape(3, CT, 128, 2 * C)
    w = w.transpose(1, 0, 2, 3)                      # [ci, t, p, co]
    wtq = np.ascontiguousarray(
        w[:, :, :, :C].transpose(2, 0, 1, 3)).reshape(128, 6 * 256).astype(BF)
    wtk = np.ascontiguousarray(
        w[:, :, :, C:].transpose(2, 0, 1, 3)).reshape(128, 6 * 256).astype(BF)
    bqT = conv_b[:C].reshape(CT, 128).T              # [128, CT]
    bqa = np.ascontiguousarray(
        np.concatenate([bqT, bqT + 1.0], axis=1)).astype(np.float32)
    obk = np.ones((1, 384), dtype=BF)
    obk[0, 128:384] = conv_b[C:].astype(BF)
    return xb, xt, wtk, wtq, bqa, obk


def make_in_maps(x, conv_w, conv_b):
    xb, xt, wtk, wtq, bqa, obk = _prep(x, conv_w, conv_b)
    return [
        {"xb": xb[b], "xt": xt[b], "wtk": wtk, "wtq": wtq,
         "bqa": bqa, "obk": obk}
        for b in range(B)
    ]


def kernel(x: np.ndarray, conv_w: np.ndarray, conv_b: np.ndarray) -> np.ndarray:
    nc = _get_nc()
    in_maps = make_in_maps(x, conv_w, conv_b)
    res = run_bass_kernel_spmd(nc, in_maps, core_ids=list(range(NCORES)))
    return np.stack([res.results[b]["out"] for b in range(B)],
                    axis=0).astype(np.float32)


# revision 26
# speedup vs baseline: 2.1979x; 1.0114x over previous
"""Trainium2 Bass kernel for nn_AttentionLayer (conv1d -> linear attention -> gelu + residual).

Full inputs:  x [8, 256, 4096] f32, conv_w [512, 256, 3] f32, conv_b [512] f32
Full output:  [8, 256, 4096] f32

Sharding: pure data-parallel over batch B=8 -> 8 NeuronCores, one batch each.
No collectives needed.

Per-core math (C=256, N=4096, one batch):
  y    = conv1d(x, w, pad=1) + b          # [2C, N]
  q    = phi(y[:C]),  k = phi(y[C:])      # phi = elu+1 = max(y+1, exp(min(y,0)))
  v    = x^T                              # [N, C]
  kv   = sum_n phi(k)[n,:] (x) v[n,:]     # [C, C]
  out  = gelu(q @ kv) + x                 # [C, N]

Layout trick: the conv contraction (over input channels ci) lets us produce
q in [c, n] layout (w^T as stationary operand) AND k in [n, c] layout
(x as stationary operand) with zero transposes. v^T (= x^T) is shipped
pre-transposed/pre-tiled/pre-cast from the host, as are bf16 copies of x
and w.

DMA hygiene (the real-HW bottleneck): every HWDGE dma_start costs ~650ns of
serialized ring dispatch, and scattered patterns fragment into 512B
descriptors (the naive v^T load alone was ~0.5ms on HW). So all inputs are
host-packed into layouts that give >=1KB-per-partition contiguous rows and
are loaded with a handful of large DMAs ordered so the conv-critical bytes
land first (staged wtk/xs leading chunks, v^T last); output stores are
batched into 1024-col bf16 pairs.

Matmuls run in bf16 (f32 PSUM accumulate): bf16 gets pipelined LDWEIGHTS
(f32/f32r matmuls serialize a ~107ns self-weight-load per matmul).
phi is 3 ops via the fused scalar_tensor_tensor: min (DVE) ->
exp (ACT, one table per phase) -> (y+1) max e (DVE). The conv bias for
the k half enters as a rank-1 start matmul (ones^T @ b_k); for the q half
it rides the DVE ops' per-partition scalar operand. Residual add runs on
DVE (Pool/gpsimd serialized the OUT-phase tail at ~1.1us per add).
"""

import ml_dtypes
import numpy as np

import concourse.bass as bass
import concourse.mybir as mybir
import concourse.tile as tile
from concourse import bacc
from concourse.bass_utils import run_bass_kernel_spmd

F32 = mybir.dt.float32
BF16 = mybir.dt.bfloat16
AF = mybir.ActivationFunctionType
ALU = mybir.AluOpType

B, C, N = 8, 256, 4096
NCORES = 8
CT = C // 128        # 2 c-tiles (partition groups) per 256-channel dim
NJ = N // 512        # 8 column chunks of 512
NT = N // 128        # 32 n-tiles of 128
NP = N + 2           # x padded with one zero column on each side
NH = NP // 2 + 1     # xs half-load split point (2050)

BF = ml_dtypes.bfloat16


def _build_nc(reps=1, hw_loop=False):
    nc = bacc.Bacc("TRN2", target_bir_lowering=False, debug=False, num_devices=NCORES)

    # Host-packed parameter layouts (see _prep):
    #  xb   [CT, 128, NP]    bf16  x padded, partition-tiled over channels
    #  xt   [128, NT*256]    bf16  v^T tiled: [p, i*256+d] = x[d, i*128+p]
    #  wtk  [128, 6, 256]    bf16  conv w, k-half; slot ci*3+t, partition=cin
    #  wtq  [128, 6, 256]    bf16  conv w, q-half
    #  bqa  [128, 2*CT]      f32   [conv_b_q | conv_b_q + 1] partition-tiled
    #  obk  [1, 384]         bf16  [ones(128) | conv_b_k(256)]
    xb_d = nc.declare_dram_parameter("xb", [CT, 128, NP], BF16, isOutput=False)
    xt_d = nc.declare_dram_parameter("xt", [128, NT * 256], BF16, isOutput=False)
    wtk_d = nc.declare_dram_parameter("wtk", [128, 6 * 256], BF16, isOutput=False)
    wtq_d = nc.declare_dram_parameter("wtq", [128, 6 * 256], BF16, isOutput=False)
    bqa_d = nc.declare_dram_parameter("bqa", [128, 2 * CT], F32, isOutput=False)
    obk_d = nc.declare_dram_parameter("obk", [1, 384], BF16, isOutput=False)
    out_d = nc.declare_dram_parameter("out", [C, N], BF16, isOutput=True)

    with tile.TileContext(nc) as tc:
        with (
            tc.tile_pool(name="persist", bufs=1) as per,
            tc.tile_pool(name="tmp", bufs=6) as tmp,
            tc.tile_pool(name="obuf", bufs=3) as obuf,
            tc.tile_pool(name="psum", bufs=6, space="PSUM") as ps,
            tc.tile_pool(name="psum2", bufs=2, space="PSUM") as ps2,
        ):
          import contextlib
          loop_ctx = tc.For_i(0, reps, 1) if hw_loop else contextlib.nullcontext()
          with loop_ctx:
           for _rep in range(1 if hw_loop else reps):
            # ---- inputs: few large DMAs; SP ring for conv-critical ones ---
            obk = per.tile([1, 384], BF16, tag="obk", name="obk")
            nc.sync.dma_start(out=obk, in_=obk_d[:, :])
            ones = obk[0:1, 0:128]
            bk_sb = obk[0:1, 128:384]
            # Warm the ACT Exp table while the bulk DMAs land: the first
            # phi chain otherwise eats the ~1.3us LoadActFuncSet. (Must
            # read initialized SBUF - a memzero'd scratch tile faults the
            # exec unit.)
            warm = tmp.tile([1, 1], F32, tag="warm", name="warm")
            nc.scalar.activation(warm, obk[0:1, 0:1], AF.Exp)

            wtk = per.tile([128, 6, 256], BF16, tag="wtk", name="wtk")
            wtk_r = wtk_d.rearrange("p (s c) -> p s c", c=256)
            nc.sync.dma_start(out=wtk[:, 0:3, :], in_=wtk_r[:, 0:3, :])

            # x in [c, n] layout: one tile per ci, staged loads (small
            # leading chunk per ci so the conv starts ASAP, then the rest).
            NQ = 514
            xs = [per.tile([128, NP], BF16, tag=f"x{ci}", name=f"x{ci}")
                  for ci in range(CT)]
            nc.sync.dma_start(out=xs[0][:, 0:NQ], in_=xb_d[0, :, 0:NQ])
            nc.sync.dma_start(out=wtk[:, 3:6, :], in_=wtk_r[:, 3:6, :])
            nc.sync.dma_start(out=xs[1][:, 0:NQ], in_=xb_d[1, :, 0:NQ])
            for ci in range(CT):
                nc.sync.dma_start(out=xs[ci][:, NQ:NH], in_=xb_d[ci, :, NQ:NH])
            bqa = per.tile([128, 2 * CT], F32, tag="bqa", name="bqa")
            nc.sync.dma_start(out=bqa, in_=bqa_d[:, :])
            for ci in range(CT):
                nc.sync.dma_start(out=xs[ci][:, NH:NP], in_=xb_d[ci, :, NH:NP])

            wtq = per.tile([128, 6, 256], BF16, tag="wtq", name="wtq")
            nc.sync.dma_start(out=wtq, in_=wtq_d.rearrange("p (s c) -> p s c", c=256))

            # v^T last: not needed until phase KV, and its 2MB transfer
            # must not delay the conv-critical loads above.
            vT = per.tile([128, NT, 256], BF16, tag="vT", name="vT")
            nc.sync.dma_start(
                out=vT, in_=xt_d.rearrange("p (i d) -> p i d", d=256))

            # ---- persistent intermediates --------------------------------
            kT = per.tile([128, NT, 256], BF16, tag="kT", name="kT")
            qphi = [per.tile([128, N], BF16, tag=f"qphi{ct}", name=f"qphi{ct}")
                    for ct in range(CT)]
            kv_sb = per.tile([128, CT, 256], BF16, tag="kv", name="kv_sb")

            # ---- phase NT: k^T (conv in transposed layout) ---------------
            for i in range(NT):
                off = i * 128
                kt_ps = ps.tile([128, 512], F32, tag="bank", name="kt_ps")
                # bias row: ones^T @ bk broadcasts conv_b[k-half] over rows
                # (walrus rejects non-PE engines preloading PSUM, so this
                # stays a rank-1 start matmul)
                kt_ps = kt_ps[:, 0:256]
                nc.tensor.matmul(kt_ps, ones, bk_sb, start=True, stop=False)
                for ci in range(CT):
                    for t in range(3):
                        nc.tensor.matmul(
                            kt_ps,
                            xs[ci][:, off + t:off + t + 128],
                            wtk[:, ci * 3 + t, :],
                            start=False,
                            stop=(ci == CT - 1 and t == 2),
                        )
                # phi: kT = max(y+1, exp(min(y, 0)))
                tmin = tmp.tile([128, 256], F32, tag="ntmin", name="tmin_nt")
                nc.vector.tensor_scalar(tmin, kt_ps, 0.0, None, ALU.min)
                e = tmp.tile([128, 256], F32, tag="nte", name="e_nt")
                nc.scalar.activation(e, tmin, AF.Exp)
                nc.vector.scalar_tensor_tensor(
                    kT[:, i, :], kt_ps, 1.0, e, ALU.add, ALU.max)

            # ---- phase Q: conv q in [c, n] layout ------------------------
            for ct in range(CT):
                for j in range(NJ):
                    q_ps = ps.tile([128, 512], F32, tag="bank", name="q_ps")
                    first = True
                    for ci in range(CT):
                        for t in range(3):
                            nc.tensor.matmul(
                                q_ps,
                                wtq[:, ci * 3 + t, ct * 128:(ct + 1) * 128],
                                xs[ci][:, j * 512 + t:j * 512 + t + 512],
                                start=first,
                                stop=(ci == CT - 1 and t == 2),
                            )
                            first = False
                    # phi with per-partition conv bias folded in:
                    #   min(y+b, 0) then (y + (b+1)) max exp(...)
                    tmin = tmp.tile([128, 512], F32, tag="qtmin", name="tmin_q")
                    nc.vector.tensor_scalar(
                        tmin, q_ps, bqa[:, ct:ct + 1], 0.0, ALU.add, ALU.min)
                    e = tmp.tile([128, 512], F32, tag="qte", name="e_q")
                    nc.scalar.activation(e, tmin, AF.Exp)
                    nc.vector.scalar_tensor_tensor(
                        qphi[ct][:, j * 512:(j + 1) * 512],
                        q_ps, bqa[:, CT + ct:CT + ct + 1], e, ALU.add, ALU.max)

            # ---- phase KV: kv[c, d] = sum_n k^T[n, c] v^T[n, d] ----------
            for ch in range(CT):
                kv_ps = ps2.tile([128, 256], F32, tag="kvp", name="kv_ps")
                for i in range(NT):
                    nc.tensor.matmul(
                        kv_ps,
                        kT[:, i, ch * 128:(ch + 1) * 128],
                        vT[:, i, :],
                        start=(i == 0),
                        stop=(i == NT - 1),
                    )
                nc.vector.tensor_copy(kv_sb[:, ch, :], kv_ps)

            # ---- phase OUT: out[d, n] = gelu(sum_c kv[c, d] q[c, n]) + x -
            # Stores batched in 1024-col pairs: halves the HWDGE dispatches
            # while keeping the tail transfer small.
            for dt in range(CT):
                ob = None
                for j in range(NJ):
                    o_ps = ps.tile([128, 512], F32, tag="bank", name="o_ps")
                    for ch in range(CT):
                        nc.tensor.matmul(
                            o_ps,
                            kv_sb[:, ch, dt * 128:(dt + 1) * 128],
                            qphi[ch][:, j * 512:(j + 1) * 512],
                            start=(ch == 0),
                            stop=(ch == CT - 1),
                        )
                    g = tmp.tile([128, 512], BF16, tag="og", name="g_out")
                    nc.scalar.activation(g, o_ps, AF.Gelu)
                    if j % 2 == 0:
                        ob = obuf.tile([128, 1024], BF16, tag="ob", name="ob")
                    nc.vector.tensor_tensor(
                        ob[:, (j % 2) * 512:(j % 2) * 512 + 512],
                        g, xs[dt][:, j * 512 + 1:j * 512 + 513], ALU.add)
                    if j % 2 == 1:
                        nc.sync.dma_start(
                            out=out_d[dt * 128:(dt + 1) * 128,
                                      (j - 1) * 512:(j + 1) * 512],
                            in_=ob,
                        )

    nc.compile()
    return nc


_NC_CACHE = None


def _get_nc():
    global _NC_CACHE
    if _NC_CACHE is None:
        _NC_CACHE = _build_nc()
    return _NC_CACHE


def _prep(x, conv_w, conv_b):
    x = np.asarray(x, dtype=np.float32)
    conv_w = np.asarray(conv_w, dtype=np.float32)
    conv_b = np.asarray(conv_b, dtype=np.float32)
    xb = np.zeros((B, CT, 128, NP), dtype=BF)
    xb[:, :, :, 1:N + 1] = x.reshape(B, CT, 128, N).astype(BF)
    # xt[b, p, i*256 + d] = x[b, d, i*128 + p]  (v^T tiled for contiguous DMA)
    xt = np.ascontiguousarray(
        x.transpose(0, 2, 1).reshape(B, NT, 128, C).transpose(0, 2, 1, 3)
    ).reshape(B, 128, NT * C).astype(BF)
    # w[t, ci_t, p, co] = conv_w[co, ci_t*128 + p, t]; slot = ci*3 + t
    w = conv_w.transpose(2, 1, 0).reshape(3, CT, 128, 2 * C)
    w = w.transpose(1, 0, 2, 3)                      # [ci, t, p, co]
    wtq = np.ascontiguousarray(
        w[:, :, :, :C].transpose(2, 0, 1, 3)).reshape(128, 6 * 256).astype(BF)
    wtk = np.ascontiguousarray(
        w[:, :, :, C:].transpose(2, 0, 1, 3)).reshape(128, 6 * 256).astype(BF)
    bqT = conv_b[:C].reshape(CT, 128).T              # [128, CT]
    bqa = np.ascontiguousarray(
        np.concatenate([bqT, bqT + 1.0], axis=1)).astype(np.float32)
    obk = np.ones((1, 384), dtype=BF)
    obk[0, 128:384] = conv_b[C:].astype(BF)
    return xb, xt, wtk, wtq, bqa, obk


def make_in_maps(x, conv_w, conv_b):
    xb, xt, wtk, wtq, bqa, obk = _prep(x, conv_w, conv_b)
    return [
        {"xb": xb[b], "xt": xt[b], "wtk": wtk, "wtq": wtq,
         "bqa": bqa, "obk": obk}
        for b in range(B)
    ]


def kernel(x: np.ndarray, conv_w: np.ndarray, conv_b: np.ndarray) -> np.ndarray:
    nc = _get_nc()
    in_maps = make_in_maps(x, conv_w, conv_b)
    res = run_bass_kernel_spmd(nc, in_maps, core_ids=list(range(NCORES)))
    return np.stack([res.results[b]["out"] for b in range(B)],
                    axis=0).astype(np.float32)
